# revision 15
# baseline (speedup 1.0000x reference)
"""Paged decode attention + cache update, distributed over 8 TRN2 NeuronCores.

Strategy (unit-parallel with truncation, bf16 compute):
- A unit = one (sequence, kv-head) pair; 256 units total. Units are sorted by
  cache position and dealt round-robin across the 8 cores, so graph slot j
  holds 8 near-equal-length units and gets a static slot budget
  budget[j] = max of those 8 positions. Only cache positions below each
  sequence's pos are shipped/computed: slot-exact for K, page-granular for V.
- Host packs, per core, K transposed to [D, slots] and V native [S, pages*D],
  both bf16, as flat arrays with static per-slot offsets (the same graph runs
  SPMD on all 8 cores). K loads issue on the Sync HWDGE, V loads on the
  Scalar HWDGE. Units are processed in an interleaved big/small order so the
  DMA pipeline stays fed through the small-unit tail.
- Device per core, per unit: score matmuls (lhsT = KT chunk, rhs = qT cols)
  -> PSUM scoresT [slot, chunk*G+g]; ACT exp straight from PSUM (no
  max-subtraction -- scores have std ~1); causal masking applied
  multiplicatively AFTER exp via an on-device iota-vs-pos compare; ones-matmul
  for the softmax denominator; V matmuls accumulate out [D, G] over chunks
  (partial last chunk contracts only the valid rows).
- The new token's contribution and final normalization are rank-1 host terms,
  as is the cache scatter-update.
"""

import numpy as np

B, H, HKV, D = 32, 32, 8, 128
P, S = 32, 128          # pages per sequence, slots per page
L = P * S
G = H // HKV            # GQA group = 4
NCORES = 8
U = 32                  # unit slots per core (B*HKV/NCORES)
SCALE = 1.0 / np.sqrt(D)

_COMPILED = {}


def _plan(pos):
    """Static schedule from cache positions: per-slot budgets + unit deal."""
    pu = np.repeat(pos, HKV)
    order = np.argsort(-pu, kind="stable")
    budgets = tuple(int(pu[order[8 * j]]) for j in range(U))
    assign = order.reshape(U, NCORES)   # core i, slot j <- unit assign[j, i]
    return budgets, assign


def _ceil(n):
    return -(-n // S)


def _build(budgets):
    from contextlib import ExitStack

    import concourse.bass as bass
    import concourse.mybir as mybir
    import concourse.tile as tile
    from concourse import bacc

    f32 = mybir.dt.float32
    bf16 = mybir.dt.bfloat16
    i32 = mybir.dt.int32
    nc = bacc.Bacc()

    koffs = np.concatenate([[0], np.cumsum([n for n in budgets])])
    voffs = np.concatenate([[0], np.cumsum([_ceil(n) * D for n in budgets])])
    doffs = np.concatenate([[0], np.cumsum([_ceil(n) * G for n in budgets])])
    ktot, vtot, dtot = int(koffs[-1]), int(voffs[-1]), int(doffs[-1])

    qt_d = nc.dram_tensor("qt", (D, U * G), bf16, kind="ExternalInput")
    kt_d = nc.dram_tensor("kt", (ktot * D,), bf16, kind="ExternalInput")
    v_d = nc.dram_tensor("v", (vtot * S,), bf16, kind="ExternalInput")
    pos_d = nc.dram_tensor("posv", (S, U), f32, kind="ExternalInput")
    o_d = nc.dram_tensor("o", (D, U * G), f32, kind="ExternalOutput")
    den_d = nc.dram_tensor("den", (1, dtot), f32, kind="ExternalOutput")

    order = list(range(U))

    with tile.TileContext(nc) as tc, ExitStack() as ctx:
        const = ctx.enter_context(tc.tile_pool(name="const", bufs=1))
        kpool = ctx.enter_context(tc.tile_pool(name="kpool", bufs=8))
        vpool = ctx.enter_context(tc.tile_pool(name="vpool", bufs=8))
        wpool = ctx.enter_context(tc.tile_pool(name="wpool", bufs=3))
        spool = ctx.enter_context(tc.tile_pool(name="spool", bufs=3))
        psc = ctx.enter_context(
            tc.tile_pool(name="psc", bufs=4, space=bass.MemorySpace.PSUM)
        )
        po = ctx.enter_context(
            tc.tile_pool(name="po", bufs=2, space=bass.MemorySpace.PSUM)
        )
        pd = ctx.enter_context(
            tc.tile_pool(name="pd", bufs=2, space=bass.MemorySpace.PSUM)
        )

        qt_t = const.tile([D, U * G], bf16)
        nc.gpsimd.dma_start(qt_t[:], qt_d[:])
        pos_t = const.tile([S, U], f32)
        nc.gpsimd.dma_start(pos_t[:], pos_d[:])
        ones_t = const.tile([S, 1], bf16)
        nc.gpsimd.memset(ones_t[:], 1.0)
        # iota[s, c*G+g] = c*S + s (logical cache position of that column)
        iota_i = const.tile([S, P * G], i32)
        nc.gpsimd.iota(iota_i[:], [[S, P], [0, G]], channel_multiplier=1)
        iota_f = const.tile([S, P * G], f32)
        nc.vector.tensor_copy(iota_f[:], iota_i[:])

        o_all = const.tile([D, U * G], f32, tag="o_all")
        den_all = const.tile([1, dtot], f32, tag="den_all")

        for j in order:
            n = budgets[j]
            if n == 0:
                continue
            c = _ceil(n)
            ko, vo, do = int(koffs[j]), int(voffs[j]), int(doffs[j])
            kt_t = kpool.tile([D, c * S], bf16, tag="kt")
            nc.sync.dma_start(
                kt_t[:, :n],
                kt_d[ko * D : (ko + n) * D].rearrange("(d f) -> d f", d=D),
            )
            if n < c * S:
                nc.vector.memset(kt_t[:, n : c * S], 0.0)
            v_t = vpool.tile([S, c * D], bf16, tag="vt")
            nc.scalar.dma_start(
                v_t[:],
                v_d[vo * S : (vo + c * D) * S].rearrange(
                    "(s f) -> s f", s=S
                ),
            )

            sc_ps = psc.tile([S, c * G], f32, tag="sc")
            for t in range(c):
                nc.tensor.matmul(
                    sc_ps[:, t * G : (t + 1) * G],
                    kt_t[:, t * S : (t + 1) * S],
                    qt_t[:, j * G : (j + 1) * G],
                    start=True,
                    stop=True,
                )
            w_raw = wpool.tile([S, c * G], bf16, tag="w_raw")
            nc.scalar.activation(
                w_raw[:], sc_ps[:], mybir.ActivationFunctionType.Exp
            )
            m_t = spool.tile([S, c * G], bf16, tag="m")
            nc.vector.tensor_scalar(
                m_t[:],
                iota_f[:, : c * G],
                pos_t[:, j : j + 1],
                None,
                mybir.AluOpType.is_lt,
            )
            w_t = wpool.tile([S, c * G], bf16, tag="w")
            nc.vector.tensor_mul(w_t[:], w_raw[:], m_t[:])

            den_ps = pd.tile([1, c * G], f32, tag="den")
            nc.tensor.matmul(den_ps[:], ones_t[:], w_t[:], start=True, stop=True)

            o_ps = po.tile([D, G], f32, tag="o")
            for t in range(c):
                nc.tensor.matmul(
                    o_ps[:],
                    v_t[:, t * D : (t + 1) * D],
                    w_t[:, t * G : (t + 1) * G],
                    start=(t == 0),
                    stop=(t == c - 1),
                )

            nc.vector.tensor_copy(o_all[:, j * G : (j + 1) * G], o_ps[:])
            nc.vector.tensor_copy(den_all[:, do : do + c * G], den_ps[:])

        nc.sync.dma_start(o_d[:], o_all[:])
        nc.scalar.dma_start(den_d[:], den_all[:])

    nc.compile()
    return nc


def _get_nc(budgets):
    if budgets not in _COMPILED:
        _COMPILED[budgets] = _build(budgets)
    return _COMPILED[budgets]


def kernel(query, key, value, k_cache, v_cache, cache_position, page_table):
    import ml_dtypes

    from concourse.bass_utils import run_bass_kernel_spmd

    bf16 = ml_dtypes.bfloat16
    query = np.asarray(query, dtype=np.float32)
    key = np.asarray(key, dtype=np.float32)
    value = np.asarray(value, dtype=np.float32)
    k_cache = np.asarray(k_cache, dtype=np.float32)
    v_cache = np.asarray(v_cache, dtype=np.float32)
    pos = np.asarray(cache_position, dtype=np.int64)
    pt = np.asarray(page_table, dtype=np.int64)

    budgets, assign = _plan(pos)
    nc = _get_nc(budgets)
    _COMPILED["last_nc"] = nc

    koffs = np.concatenate([[0], np.cumsum([n for n in budgets])])
    voffs = np.concatenate([[0], np.cumsum([_ceil(n) * D for n in budgets])])
    doffs = np.concatenate([[0], np.cumsum([_ceil(n) * G for n in budgets])])
    ktot, vtot, dtot = int(koffs[-1]), int(voffs[-1]), int(doffs[-1])
    qg = (query.reshape(B, HKV, G, D) * SCALE).astype(bf16)

    in_maps = []
    for i in range(NCORES):
        ktf = np.zeros((D, ktot), dtype=bf16)
        vvf = np.zeros((S, vtot), dtype=bf16)
        qt = np.zeros((D, U * G), dtype=bf16)
        posv = np.zeros((S, U), dtype=np.float32)
        for j in range(U):
            n = budgets[j]
            if n == 0:
                continue
            c = _ceil(n)
            uid = assign[j, i]
            b, h = divmod(int(uid), HKV)
            nv = int(pos[b])                     # valid cache positions
            ko, vo = int(koffs[j]), int(voffs[j])
            if nv:
                npg = _ceil(nv)
                pages = pt[b][:npg]
                kj = (
                    k_cache[pages, h].reshape(npg * S, D)[:nv].astype(bf16)
                )
                vj = (
                    v_cache[pages, h].reshape(npg * S, D)[:nv].astype(bf16)
                )
                ktf[:, ko : ko + nv] = kj.T
                vp = np.zeros((c * S, D), dtype=bf16)
                vp[:nv] = vj
                vvf[:, vo : vo + c * D] = (
                    vp.reshape(c, S, D).transpose(1, 0, 2).reshape(S, c * D)
                )
            qt[:, j * G : (j + 1) * G] = qg[b, h].T
            posv[:, j] = float(nv)
        # flat layouts: per-unit [D, n] / [S, c*D] rectangles, concatenated
        kt_flat = np.concatenate(
            [
                ktf[:, int(koffs[j]) : int(koffs[j + 1])].reshape(-1)
                for j in range(U)
                if budgets[j] > 0
            ]
        ) if ktot else np.zeros(0, dtype=bf16)
        v_flat = np.concatenate(
            [
                vvf[:, int(voffs[j]) : int(voffs[j + 1])].reshape(-1)
                for j in range(U)
                if budgets[j] > 0
            ]
        ) if vtot else np.zeros(0, dtype=bf16)
        in_maps.append(
            {"qt": qt, "kt": kt_flat, "v": v_flat, "posv": posv}
        )

    _COMPILED["in_maps"] = in_maps
    res = run_bass_kernel_spmd(nc, in_maps, core_ids=list(range(NCORES)))
    outs = res.results

    out_bhg = np.zeros((B, HKV, G, D), dtype=np.float64)
    den_sum = np.zeros((B, HKV, G), dtype=np.float64)
    for i in range(NCORES):
        o = outs[i]["o"]          # [D, U*G]
        den = outs[i]["den"]      # [1, dtot]
        for j in range(U):
            n = budgets[j]
            if n == 0:
                continue
            c = _ceil(n)
            do = int(doffs[j])
            uid = assign[j, i]
            b, h = divmod(int(uid), HKV)
            out_bhg[b, h] = o[:, j * G : (j + 1) * G].T
            den_sum[b, h] = den[0, do : do + c * G].reshape(c, G).sum(0)

    # new-token contribution (host rank-1 term)
    qgf = query.reshape(B, HKV, G, D)
    s_new = np.einsum("bkgd,bkd->bkg", qgf, key[:, :, 0, :]) * SCALE
    w_new = np.exp(s_new)                                       # [B, HKV, G]
    num = out_bhg + w_new[..., None] * value[:, :, 0, :][:, :, None, :]
    out = (num / (den_sum + w_new)[..., None]).reshape(B, H, 1, D)

    # cache update (host scatter)
    kc = np.array(k_cache)
    vc = np.array(v_cache)
    phys = pt[np.arange(B), pos // S]
    slot = pos % S
    kc[phys, :, slot, :] = key[:, :, 0, :]
    vc[phys, :, slot, :] = value[:, :, 0, :]

    return out.astype(np.float32), kc, vc


# revision 16
# speedup vs baseline: 1.0404x; 1.0404x over previous
"""Paged decode attention + cache update, distributed over 8 TRN2 NeuronCores.

Strategy (unit-parallel with truncation, bf16 compute):
- A unit = one (sequence, kv-head) pair; 256 units total. Units are sorted by
  sequence length and dealt round-robin across the 8 cores, so graph slot j
  holds 8 near-equal-length units and gets a static page budget
  budget[j] = max of those 8 lengths. Only pages below each sequence's cache
  position are shipped/computed (~57% of the full cache here).
- Host packs, per core, the K pages transposed to [D, slot] and V pages
  native, both bf16, as flat arrays with static per-slot offsets (the same
  graph runs SPMD on all 8 cores). K loads issue on the Sync HWDGE, V loads
  on the Scalar HWDGE to keep both DMA issue streams fed.
- Device per core, per unit slot j (budget n): n score matmuls
  (lhsT = KT page, rhs = qT[:, 4j:4j+4]) -> PSUM scoresT [slot, p*G+g];
  ACT exp straight from PSUM (no max-subtraction -- scores have std ~1);
  causal masking is applied multiplicatively AFTER exp with an on-device
  iota-vs-position compare (saves all mask DMA traffic); ones-matmul gives
  the softmax denominator; n V matmuls accumulate out [D, G] over pages.
- The new token's contribution and final normalization are rank-1 terms
  folded in on the host, as is the cache scatter-update.
"""

import numpy as np

B, H, HKV, D = 32, 32, 8, 128
P, S = 32, 128          # pages per sequence, slots per page
L = P * S
G = H // HKV            # GQA group = 4
NCORES = 8
U = 32                  # unit slots per core (B*HKV/NCORES)
SCALE = 1.0 / np.sqrt(D)
NEG = -1e9

_COMPILED = {}


def _plan(pos):
    """Static schedule from cache positions: per-slot budgets + unit deal."""
    n_pages = -(-pos // S)              # valid cache pages per sequence
    units = np.repeat(n_pages, HKV)     # unit id = b*HKV + h
    order = np.argsort(-units, kind="stable")
    budgets = tuple(int(units[order[8 * j]]) for j in range(U))
    assign = order.reshape(U, NCORES)   # core i, slot j <- unit assign[j, i]
    offs = np.concatenate([[0], np.cumsum(budgets)]).astype(np.int64)
    return budgets, assign, offs


def _build(budgets):
    from contextlib import ExitStack

    import concourse.bass as bass
    import concourse.mybir as mybir
    import concourse.tile as tile
    from concourse import bacc

    f32 = mybir.dt.float32
    bf16 = mybir.dt.bfloat16
    i32 = mybir.dt.int32
    nc = bacc.Bacc()
    tot = sum(budgets)
    dtot = sum(n * G for n in budgets)

    qt_d = nc.dram_tensor("qt", (D, U * G), bf16, kind="ExternalInput")
    kt_d = nc.dram_tensor("kt", (tot * D * S,), bf16, kind="ExternalInput")
    v_d = nc.dram_tensor("v", (tot * S * D,), bf16, kind="ExternalInput")
    pos_d = nc.dram_tensor("posv", (S, U), f32, kind="ExternalInput")
    o_d = nc.dram_tensor("o", (D, U * G), f32, kind="ExternalOutput")
    den_d = nc.dram_tensor("den", (1, dtot), f32, kind="ExternalOutput")

    with tile.TileContext(nc) as tc, ExitStack() as ctx:
        const = ctx.enter_context(tc.tile_pool(name="const", bufs=1))
        kpool = ctx.enter_context(tc.tile_pool(name="kpool", bufs=8))
        vpool = ctx.enter_context(tc.tile_pool(name="vpool", bufs=8))
        wpool = ctx.enter_context(tc.tile_pool(name="wpool", bufs=3))
        spool = ctx.enter_context(tc.tile_pool(name="spool", bufs=3))
        psc = ctx.enter_context(
            tc.tile_pool(name="psc", bufs=4, space=bass.MemorySpace.PSUM)
        )
        po = ctx.enter_context(
            tc.tile_pool(name="po", bufs=2, space=bass.MemorySpace.PSUM)
        )
        pd = ctx.enter_context(
            tc.tile_pool(name="pd", bufs=2, space=bass.MemorySpace.PSUM)
        )

        qt_t = const.tile([D, U * G], bf16)
        nc.gpsimd.dma_start(qt_t[:], qt_d[:])
        pos_t = const.tile([S, U], f32)
        nc.gpsimd.dma_start(pos_t[:], pos_d[:])
        ones_t = const.tile([S, 1], bf16)
        nc.gpsimd.memset(ones_t[:], 1.0)
        # iota[s, p*G+g] = p*S + s (logical cache position of that column)
        iota_i = const.tile([S, P * G], i32)
        nc.gpsimd.iota(iota_i[:], [[S, P], [0, G]], channel_multiplier=1)
        iota_f = const.tile([S, P * G], f32)
        nc.vector.tensor_copy(iota_f[:], iota_i[:])

        o_all = const.tile([D, U * G], f32, tag="o_all")
        den_all = const.tile([1, dtot], f32, tag="den_all")

        off = 0
        doff = 0
        for j in range(U):
            n = budgets[j]
            if n == 0:
                continue
            kt_t = kpool.tile([D, n * S], bf16, tag="kt")
            nc.sync.dma_start(
                kt_t[:],
                kt_d[off * D * S : (off + n) * D * S].rearrange(
                    "(d f) -> d f", d=D
                ),
            )
            v_t = vpool.tile([S, n * D], bf16, tag="vt")
            nc.scalar.dma_start(
                v_t[:],
                v_d[off * S * D : (off + n) * S * D].rearrange(
                    "(s f) -> s f", s=S
                ),
            )

            sc_ps = psc.tile([S, n * G], f32, tag="sc")
            for p in range(n):
                nc.tensor.matmul(
                    sc_ps[:, p * G : (p + 1) * G],
                    kt_t[:, p * S : (p + 1) * S],
                    qt_t[:, j * G : (j + 1) * G],
                    start=True,
                    stop=True,
                )
            w_raw = wpool.tile([S, n * G], bf16, tag="w_raw")
            nc.scalar.activation(
                w_raw[:], sc_ps[:], mybir.ActivationFunctionType.Exp
            )
            m_t = spool.tile([S, n * G], bf16, tag="m")
            nc.vector.tensor_scalar(
                m_t[:],
                iota_f[:, : n * G],
                pos_t[:, j : j + 1],
                None,
                mybir.AluOpType.is_lt,
            )
            w_t = wpool.tile([S, n * G], bf16, tag="w")
            nc.vector.tensor_mul(w_t[:], w_raw[:], m_t[:])

            den_ps = pd.tile([1, n * G], f32, tag="den")
            nc.tensor.matmul(den_ps[:], ones_t[:], w_t[:], start=True, stop=True)

            o_ps = po.tile([D, G], f32, tag="o")
            for p in range(n):
                nc.tensor.matmul(
                    o_ps[:],
                    v_t[:, p * D : (p + 1) * D],
                    w_t[:, p * G : (p + 1) * G],
                    start=(p == 0),
                    stop=(p == n - 1),
                )

            nc.vector.tensor_copy(o_all[:, j * G : (j + 1) * G], o_ps[:])
            nc.vector.tensor_copy(
                den_all[:, doff : doff + n * G], den_ps[:]
            )
            off += n
            doff += n * G

        nc.gpsimd.dma_start(o_d[:], o_all[:])
        nc.gpsimd.dma_start(den_d[:], den_all[:])

    nc.compile()
    return nc


def _get_nc(budgets):
    if budgets not in _COMPILED:
        _COMPILED[budgets] = _build(budgets)
    return _COMPILED[budgets]


def kernel(query, key, value, k_cache, v_cache, cache_position, page_table):
    import ml_dtypes

    from concourse.bass_utils import run_bass_kernel_spmd

    bf16 = ml_dtypes.bfloat16
    query = np.asarray(query, dtype=np.float32)
    key = np.asarray(key, dtype=np.float32)
    value = np.asarray(value, dtype=np.float32)
    k_cache = np.asarray(k_cache, dtype=np.float32)
    v_cache = np.asarray(v_cache, dtype=np.float32)
    pos = np.asarray(cache_position, dtype=np.int64)
    pt = np.asarray(page_table, dtype=np.int64)

    budgets, assign, offs = _plan(pos)
    tot = int(sum(budgets))
    dtot = int(sum(n * G for n in budgets))
    nc = _get_nc(budgets)
    _COMPILED["last_nc"] = nc

    qg = (query.reshape(B, HKV, G, D) * SCALE).astype(bf16)

    in_maps = []
    for i in range(NCORES):
        kt = np.zeros(tot * D * S, dtype=bf16)
        vv = np.zeros(tot * S * D, dtype=bf16)
        qt = np.zeros((D, U * G), dtype=bf16)
        posv = np.zeros((S, U), dtype=np.float32)
        for j in range(U):
            nb = budgets[j]
            if nb == 0:
                continue
            uid = assign[j, i]
            b, h = divmod(int(uid), HKV)
            nv = int(-(-pos[b] // S))            # valid pages for this seq
            pages = pt[b][:nv]
            o0 = int(offs[j])
            kj = k_cache[pages, h].astype(bf16)  # [nv, S, D]
            vj = v_cache[pages, h].astype(bf16)
            ktj = np.zeros((D, nb * S), dtype=bf16)
            ktj[:, : nv * S] = kj.transpose(2, 0, 1).reshape(D, nv * S)
            vvj = np.zeros((S, nb * D), dtype=bf16)
            vvj[:, : nv * D] = vj.transpose(1, 0, 2).reshape(S, nv * D)
            kt[o0 * D * S : (o0 + nb) * D * S] = ktj.reshape(-1)
            vv[o0 * S * D : (o0 + nb) * S * D] = vvj.reshape(-1)
            qt[:, j * G : (j + 1) * G] = qg[b, h].T
            posv[:, j] = float(pos[b])
        in_maps.append({"qt": qt, "kt": kt, "v": vv, "posv": posv})

    _COMPILED["in_maps"] = in_maps
    res = run_bass_kernel_spmd(nc, in_maps, core_ids=list(range(NCORES)))
    outs = res.results

    out_bhg = np.zeros((B, HKV, G, D), dtype=np.float64)
    den_sum = np.zeros((B, HKV, G), dtype=np.float64)
    for i in range(NCORES):
        o = outs[i]["o"]          # [D, U*G]
        den = outs[i]["den"]      # [1, dtot]
        doff = 0
        for j in range(U):
            nb = budgets[j]
            if nb == 0:
                continue
            uid = assign[j, i]
            b, h = divmod(int(uid), HKV)
            out_bhg[b, h] = o[:, j * G : (j + 1) * G].T
            den_sum[b, h] = den[0, doff : doff + nb * G].reshape(nb, G).sum(0)
            doff += nb * G

    # new-token contribution (host rank-1 term)
    qgf = query.reshape(B, HKV, G, D)
    s_new = np.einsum("bkgd,bkd->bkg", qgf, key[:, :, 0, :]) * SCALE
    w_new = np.exp(s_new)                                       # [B, HKV, G]
    num = out_bhg + w_new[..., None] * value[:, :, 0, :][:, :, None, :]
    out = (num / (den_sum + w_new)[..., None]).reshape(B, H, 1, D)

    # cache update (host scatter)
    kc = np.array(k_cache)
    vc = np.array(v_cache)
    phys = pt[np.arange(B), pos // S]
    slot = pos % S
    kc[phys, :, slot, :] = key[:, :, 0, :]
    vc[phys, :, slot, :] = value[:, :, 0, :]

    return out.astype(np.float32), kc, vc
